# revision 1
# baseline (speedup 1.0000x reference)
"""Trainium2 Bass kernel for CoocOpModel.

out[b,s,z] = sum_{i,j} func[b,s,i] * cooc[i,j,z] * arg[b,s,j]
  with func = func_and_arg[..., :128], arg = func_and_arg[..., 128:]

Shapes (hardcoded): func_and_arg [4,1024,256] f32, cooccurrences [128,128,128] f32,
out [4,1024,128] f32.  D = 128, tokens T = 4096.

Strategy: data-parallel over tokens across 8 cores (512 tokens/core);
cooccurrence tensor replicated per core (fp16).

Per-core math, with t = local token index (512), i/j/z in [0,128):
  out_T[z, t] = sum_i  C_i^T @ G_i        (accumulated in one PSUM bank)
  C_i[j, z]   = cooc[i, j, z]             (stationary operand, fp16)
  G_i[j, t]   = arg_T[j, t] * func_T[i, t]  (moving operand, fp16)

i's are processed in groups of GRP=8:
  - one broadcast-DMA materializes f_exp_g[j, (k,t)] = func_T[8g+k, t]
    (replicated across the 128 j-partitions; DRAM-source AP with
    partition-step 0 — SBUF sources reject step-0 partition dims)
  - one DVE tensor-tensor multiply builds G for the whole group, re-reading
    arg_T per k through a free-dim step-0 AP (no materialized a_rep)
  - 8 accumulating matmuls consume it (stationary = per-group cooc tile)
"""

import sys

sys.path.insert(0, "/opt/trn_rl_repo")

import numpy as np
import ml_dtypes
from contextlib import ExitStack

import concourse.bass as bass
import concourse.tile as tile
from concourse import bacc, mybir
from concourse.bass_utils import run_bass_kernel_spmd

BF16 = mybir.dt.float16
F32 = mybir.dt.float32
NP_BF16 = np.float16

N_CORES = 8
D = 128
T_TOTAL = 4096
T_CORE = T_TOTAL // N_CORES  # 512
GRP = 8
N_GRP = D // GRP

_NC_CACHE = None


def _build():
    nc = bacc.Bacc("TRN2", target_bir_lowering=False, debug=False, num_devices=N_CORES)

    f_t = nc.dram_tensor("f_t", [D, T_CORE], BF16, kind="ExternalInput").ap()
    a_t = nc.dram_tensor("a_t", [D, T_CORE], BF16, kind="ExternalInput").ap()
    # c2[j, i*128 + z] = cooc[i, j, z]
    c2 = nc.dram_tensor("c2", [D, D * D], BF16, kind="ExternalInput").ap()
    out_t = nc.dram_tensor("out_t", [D, T_CORE], F32, kind="ExternalOutput").ap()

    with tile.TileContext(nc) as tc:
        with ExitStack() as ctx:
            const_pool = ctx.enter_context(tc.tile_pool(name="const", bufs=1))
            fexp_pool = ctx.enter_context(tc.tile_pool(name="fexp", bufs=3))
            g_pool = ctx.enter_context(tc.tile_pool(name="g", bufs=3))
            out_pool = ctx.enter_context(tc.tile_pool(name="out", bufs=1))
            psum_pool = ctx.enter_context(
                tc.tile_pool(name="psum", bufs=1, space="PSUM")
            )

            # arg_T in SBUF; the TT re-reads it per k via a free-step-0 AP.
            a_sb = const_pool.tile([D, T_CORE], BF16, tag="a")
            nc.sync.dma_start(a_sb[:], a_t[:, :])
            a_ap = a_sb[:]

            # Group-size schedule: small head groups so the first pb-DMA
            # (which fair-shares DMA engines with the other queue's head)
            # lands fast and the TT/MM pipeline ramps early; small tail
            # groups to shorten the drain after the last big transfer.
            sizes = [GRP] * N_GRP
            assert sum(sizes) == D

            ps = psum_pool.tile([D, T_CORE], F32)
            i0 = 0
            for g, sz in enumerate(sizes):
                # f_exp[j, (k, t)] = func_T[i0+k, t], replicated over j.
                # Issued BEFORE the group's cooc tile: the f stream paces the
                # whole pipeline (TT -> MM), while cooc tiles have ~2us slack.
                f_exp = fexp_pool.tile([D, sz * T_CORE], BF16, tag="fexp")
                f_src = bass.AP(
                    f_t.tensor,
                    i0 * T_CORE,
                    [[0, D], [T_CORE, sz], [1, T_CORE]],
                )
                if g == 0:
                    # split the first broadcast across both queues so the
                    # pipeline's head transfer isn't serialized behind a
                    # full-size DMA sharing engines with the other queue
                    half = GRP // 2
                    f_src_a = bass.AP(
                        f_t.tensor, i0 * T_CORE, [[0, D], [T_CORE, half], [1, T_CORE]]
                    )
                    f_src_b = bass.AP(
                        f_t.tensor,
                        (i0 + half) * T_CORE,
                        [[0, D], [T_CORE, half], [1, T_CORE]],
                    )
                    nc.scalar.dma_start(f_exp[:, : half * T_CORE], f_src_a)
                    nc.sync.dma_start(f_exp[:, half * T_CORE :], f_src_b)
                else:
                    eng = nc.sync if g % 2 == 0 else nc.scalar
                    eng.dma_start(f_exp[:], f_src)

                # per-group cooc tile: c_sb[j, (k, z)] = cooc[i0+k, j, z]
                c_sb = const_pool.tile([D, sz * D], BF16, tag=f"c{g}")
                eng = nc.scalar if g % 2 == 0 else nc.sync
                eng.dma_start(c_sb[:], c2[:, i0 * D : (i0 + sz) * D])

                a_view = bass.AP(
                    a_ap.tensor, a_ap.offset, [a_ap.ap[0], [0, sz], [1, T_CORE]]
                )
                gt = g_pool.tile([D, sz * T_CORE], BF16, tag="g")
                if g == N_GRP - 1:
                    # split the last multiply so its first matmuls overlap
                    # the second half — trims serial tail after the final
                    # broadcast lands
                    h = sz // 2
                    a_half = bass.AP(
                        a_ap.tensor, a_ap.offset, [a_ap.ap[0], [0, h], [1, T_CORE]]
                    )
                    nc.vector.tensor_mul(
                        gt[:, : h * T_CORE], a_half, f_exp[:, : h * T_CORE]
                    )
                    nc.vector.tensor_mul(
                        gt[:, h * T_CORE :], a_half, f_exp[:, h * T_CORE :]
                    )
                else:
                    nc.vector.tensor_mul(gt[:], a_view, f_exp[:])

                for k in range(sz):
                    i = i0 + k
                    nc.tensor.matmul(
                        ps[:],
                        c_sb[:, k * D : (k + 1) * D],
                        gt[:, k * T_CORE : (k + 1) * T_CORE],
                        start=(i == 0),
                        stop=(i == D - 1),
                    )
                i0 += sz

            o_sb = out_pool.tile([D, T_CORE], F32, tag="o")
            nc.vector.tensor_copy(o_sb[:], ps[:])
            nc.sync.dma_start(out_t[:, :], o_sb[:])

    nc.compile()
    return nc


def _get_nc():
    global _NC_CACHE
    if _NC_CACHE is None:
        _NC_CACHE = _build()
    return _NC_CACHE


def _prep_in_maps(func_and_arg, cooccurrences):
    fa = np.asarray(func_and_arg, dtype=np.float32).reshape(T_TOTAL, 2 * D)
    c2 = (
        np.ascontiguousarray(
            np.asarray(cooccurrences, dtype=np.float32).transpose(1, 0, 2)
        )
        .reshape(D, D * D)
        .astype(NP_BF16)
    )
    in_maps = []
    for c in range(N_CORES):
        s = fa[c * T_CORE : (c + 1) * T_CORE]  # [512, 256]
        f_tc = np.ascontiguousarray(s[:, :D].T).astype(NP_BF16)  # [128 i, 512 t]
        a_tc = np.ascontiguousarray(s[:, D:].T).astype(NP_BF16)  # [128 j, 512 t]
        in_maps.append({"f_t": f_tc, "a_t": a_tc, "c2": c2})
    return in_maps


def kernel(func_and_arg: np.ndarray, cooccurrences: np.ndarray) -> np.ndarray:
    assert func_and_arg.shape == (4, 1024, 2 * D)
    assert cooccurrences.shape == (D, D, D)

    in_maps = _prep_in_maps(func_and_arg, cooccurrences)
    nc = _get_nc()
    res = run_bass_kernel_spmd(nc, in_maps, core_ids=list(range(N_CORES)))

    # out_t per core: [z=128, t=512] -> [t, z]; concat over cores -> [4096, 128]
    outs = [res.results[c]["out_t"].T for c in range(N_CORES)]
    out = np.concatenate(outs, axis=0).reshape(4, 1024, D).astype(np.float32)
    return out



# revision 3
# speedup vs baseline: 1.5292x; 1.5292x over previous
"""Trainium2 Bass kernel for CoocOpModel.

out[b,s,z] = sum_{i,j} func[b,s,i] * cooc[i,j,z] * arg[b,s,j]
  with func = func_and_arg[..., :128], arg = func_and_arg[..., 128:]

Shapes (hardcoded): func_and_arg [4,1024,256] f32, cooccurrences [128,128,128] f32,
out [4,1024,128] f32.  D = 128, tokens T = 4096.

Strategy: data-parallel over tokens across 8 cores (512 tokens/core).

Per-core math as ONE flattened contraction over (i,j):
  out[z, t] = sum_{(i,j)} C2[(i,j), z] * P[(i,j), t],  P[(i,j), t] = f[i,t]*a[j,t]

The 16384-long (i,j) axis is processed as 128 PSUM-accumulated matmul
chunks of 128 partition-pairs each.  A chunk covers GI=8 i's x GJ=16 j's
(partition p = ii*16 + jj).  This mixed layout is what makes the moving
operand cheap to build:

  - f slab per I-group:  f_sb[p, t] = f[I*8 + p//16, t]   (each f row
    replicated over only 16 partitions -> 16 slabs x 128KB = 2MB DMA,
    vs 16MB for a full 128-way broadcast)
  - a slabs (2 tiles):   a_all[p, J*512+t] = a[J*16 + p%16, t]  (1MB)
  - P (TT on DVE):       P[p, (J4,t)] = f_sb[p, t] * a_half[p, (J4,t)]
    with f re-read 4x through a free-dim step-0 AP.

Replication slabs are DMA'd straight from DRAM with step-0 dims
(DRAM-source APs allow partition/step-0 replication; SBUF sources don't).

PE: 128 matmuls, stationary = c2r chunk [p=128,(z)128], moving = P
[p=128, t=512], all accumulating into one PSUM bank [128z, 512t] f32.

Host pre-reorder: c2r[ii*16+jj, (I*8+J)*128 + z] = cooc[I*8+ii, J*16+jj, z].
"""

import sys

sys.path.insert(0, "/opt/trn_rl_repo")

import numpy as np
from contextlib import ExitStack

import concourse.bass as bass
import concourse.tile as tile
from concourse import bacc, mybir
from concourse.bass_utils import run_bass_kernel_spmd

F16 = mybir.dt.float16
F32 = mybir.dt.float32
NP_F16 = np.float16

N_CORES = 8
D = 128
T_TOTAL = 4096
T_CORE = T_TOTAL // N_CORES  # 512
GI, GJ = 8, 16               # i's / j's per chunk
NI, NJ = D // GI, D // GJ    # 16 I-groups, 8 J-groups

_NC_CACHE = None


def _build():
    nc = bacc.Bacc("TRN2", target_bir_lowering=False, debug=False, num_devices=N_CORES)

    f_t = nc.dram_tensor("f_t", [D, T_CORE], F16, kind="ExternalInput").ap()
    a_t = nc.dram_tensor("a_t", [D, T_CORE], F16, kind="ExternalInput").ap()
    # c2r[ii*16+jj, (I*8+J)*128 + z] = cooc[I*8+ii, J*16+jj, z]
    c2 = nc.dram_tensor("c2", [D, D * D], F16, kind="ExternalInput").ap()
    out_t = nc.dram_tensor("out_t", [D, T_CORE], F32, kind="ExternalOutput").ap()

    HALF = 4 * T_CORE  # 2048: four J-chunks per TT unit

    with tile.TileContext(nc) as tc:
        with ExitStack() as ctx:
            const_pool = ctx.enter_context(tc.tile_pool(name="const", bufs=1))
            f_pool = ctx.enter_context(tc.tile_pool(name="fsl", bufs=4))
            c_pool = ctx.enter_context(tc.tile_pool(name="csl", bufs=4))
            p_pool = ctx.enter_context(tc.tile_pool(name="p", bufs=3))
            out_pool = ctx.enter_context(tc.tile_pool(name="out", bufs=1))
            psum_pool = ctx.enter_context(
                tc.tile_pool(name="psum", bufs=1, space="PSUM")
            )

            # a replication slabs: a_half[h][p, J4*512+t] = a[(4h+J4)*16 + p%16, t]
            # (DMA APs are limited to 3 dims -> one DMA per J-chunk)
            a_halves = []
            for h in range(2):
                a_sb = const_pool.tile([D, HALF], F16, tag=f"a{h}")
                for J4 in range(4):
                    a_src = bass.AP(
                        a_t.tensor,
                        (h * 4 + J4) * GJ * T_CORE,
                        [[0, 8], [T_CORE, GJ], [1, T_CORE]],
                    )
                    nc.sync.dma_start(
                        a_sb[:, J4 * T_CORE : (J4 + 1) * T_CORE], a_src
                    )
                a_halves.append(a_sb)

            ps = psum_pool.tile([D, T_CORE], F32)

            q = 0
            for I in range(NI):
                # f slab: f_sb[p, t] = f[I*8 + p//16, t]
                f_sb = f_pool.tile([D, T_CORE], F16, tag="f")
                f_src = bass.AP(
                    f_t.tensor,
                    I * GI * T_CORE,
                    [[T_CORE, GI], [0, GJ], [1, T_CORE]],
                )
                nc.scalar.dma_start(f_sb[:], f_src)

                # cooc slab for this I-group: 8 chunks, [128, 1024] f16
                c_sb = c_pool.tile([D, NJ * D], F16, tag=f"c{I}")
                nc.sync.dma_start(c_sb[:], c2[:, I * NJ * D : (I + 1) * NJ * D])

                f_ap = f_sb[:]
                f_view = bass.AP(
                    f_ap.tensor, f_ap.offset, [f_ap.ap[0], [0, 4], [1, T_CORE]]
                )
                for h in range(2):
                    pt = p_pool.tile([D, HALF], F16, tag="p")
                    nc.vector.tensor_mul(pt[:], f_view, a_halves[h][:])
                    for J4 in range(4):
                        nc.tensor.matmul(
                            ps[:],
                            c_sb[:, (h * 4 + J4) * D : (h * 4 + J4 + 1) * D],
                            pt[:, J4 * T_CORE : (J4 + 1) * T_CORE],
                            start=(q == 0),
                            stop=(q == NI * NJ - 1),
                        )
                        q += 1

            o_sb = out_pool.tile([D, T_CORE], F32, tag="o")
            nc.scalar.copy(o_sb[:], ps[:])
            nc.scalar.dma_start(out_t[:, :], o_sb[:])

    nc.compile()
    return nc


def _get_nc():
    global _NC_CACHE
    if _NC_CACHE is None:
        _NC_CACHE = _build()
    return _NC_CACHE


def _prep_in_maps(func_and_arg, cooccurrences):
    fa = np.asarray(func_and_arg, dtype=np.float32).reshape(T_TOTAL, 2 * D)
    c2r = (
        np.asarray(cooccurrences, dtype=np.float32)
        .reshape(NI, GI, NJ, GJ, D)
        .transpose(1, 3, 0, 2, 4)
        .reshape(D, D * D)
        .astype(NP_F16)
    )
    c2r = np.ascontiguousarray(c2r)
    in_maps = []
    for c in range(N_CORES):
        s = fa[c * T_CORE : (c + 1) * T_CORE]  # [512, 256]
        f_tc = np.ascontiguousarray(s[:, :D].T).astype(NP_F16)  # [128 i, 512 t]
        a_tc = np.ascontiguousarray(s[:, D:].T).astype(NP_F16)  # [128 j, 512 t]
        in_maps.append({"f_t": f_tc, "a_t": a_tc, "c2": c2r})
    return in_maps


def kernel(func_and_arg: np.ndarray, cooccurrences: np.ndarray) -> np.ndarray:
    assert func_and_arg.shape == (4, 1024, 2 * D)
    assert cooccurrences.shape == (D, D, D)

    in_maps = _prep_in_maps(func_and_arg, cooccurrences)
    nc = _get_nc()
    res = run_bass_kernel_spmd(nc, in_maps, core_ids=list(range(N_CORES)))

    # out_t per core: [z=128, t=512] -> [t, z]; concat over cores -> [4096, 128]
    outs = [res.results[c]["out_t"].T for c in range(N_CORES)]
    out = np.concatenate(outs, axis=0).reshape(4, 1024, D).astype(np.float32)
    return out
